# revision 5
# baseline (speedup 1.0000x reference)
"""Trainium2 Bass kernel for a 6-layer transformer decoder (nn_Decoder), v2.

Sharding: data-parallel over batch across 8 NeuronCores (2 sequences/core,
weights replicated).  Host does embedding gather + positional add; device does
everything else.

v2 changes vs baseline:
  - all heavy matmul operands in bf16 (fp32 psum accumulate): halves SBUF and
    weight DMA, and avoids the fp32r small-free-dim 4x penalty.
  - LayerNorm via one-pass DVE bn_stats/bn_aggr; rstd = exp(-0.5*ln(var+eps))
    on ACT so the kernel's activation funcs {Copy,Identity,Exp,Ln,Relu} all
    live in ONE table (no 1283ns table reloads between softmax and LN).
  - causal skip in self-attention: scores matmuls restricted to q >= k-tile,
    exp writes only the live region into shared per-(b,parity) exp buffers
    whose masked regions are zeroed once per layer (GPSIMD memset); causal
    mask add is a single [P,2,128] diagonal-block DVE op per psum pair.
  - attention head loop interleaves the two batch streams for PE slack.
  - FFN restructured: w1 chunks + full w2 loaded once per layer (not per
    batch), h1 for both batches computed per chunk, y2 accumulated per
    token-tile over all 16 chunks.
  - psum->sbuf copy-outs balanced between ACT and DVE; memsets on GPSIMD.
"""

import sys

import numpy as np

# ---------------------------------------------------------------- constants
B, S, D, H, FF, L, V = 16, 512, 512, 8, 2048, 6, 32000
NCORES = 8
BPC = B // NCORES          # batches per core
T = BPC * S                # tokens per core
P = 128
DK = D // H                # 64
NT = T // P                # 8 token tiles per core
NTB = S // P               # 4 token tiles per batch
NF = D // P                # 4 feature tiles
NHC = 4                    # hidden chunks in FFN (each FF/NHC = 512 wide)
NC16 = FF // P             # 16 FF row tiles
EPS = 1e-5

_REPO = "/opt/trn_rl_repo"


def _ensure_env():
    if _REPO not in sys.path:
        sys.path.insert(0, _REPO)
    import concourse.mybir as mybir
    import concourse.tile as tile
    from bass_rust import ScopedClock

    # This container's walrus build only accepts ONE sync-wait per CTRL
    # instruction; the stock TileContext tail drain aggregates all residual
    # clock waits onto a single Drain.  Split them across multiple drains.
    if not getattr(tile.TileContext, "_ant_drain_patched", False):

        def _drain_and_barrier(self, tick_clock, wait_clock):
            nc = self.nc
            probe = nc.sync.drain()
            wait_clock.add_sem_waits(
                probe.ins, ScopedClock({None: tick_clock.global_clock})
            )
            si = probe.ins.sync_info
            waits = list(si.on_wait) if si is not None else []
            if len(waits) > 1:
                probe.ins.sync_info = mybir.SyncInfo(
                    on_wait=[waits[0]], on_update=list(si.on_update)
                )
                for w in waits[1:]:
                    extra = nc.sync.drain()
                    extra.ins.sync_info = mybir.SyncInfo(on_wait=[w], on_update=[])
            nc.all_engine_barrier()
            popped = nc._tile_sem_poison_stack.pop()
            assert popped is self._sem_poison
            nc.clear_and_free_semaphores(list(self.sems.allocated().values()))
            nc.all_engine_barrier()

        tile.TileContext._drain_and_barrier = _drain_and_barrier
        tile.TileContext._ant_drain_patched = True


def _split_multi_waits(nc):
    """Hoist extra sync-waits onto single-wait NoOps (walrus accepts only one
    wait per instruction; per-engine program order preserves gating)."""
    import concourse.mybir as mybir

    n = 0
    for f in nc.m.functions:
        for b in f.blocks:
            insts = list(b.instructions)
            out = []
            changed = False
            for inst in insts:
                si = inst.sync_info
                if si is not None and len(si.on_wait) > 1:
                    waits = list(si.on_wait)
                    for k, w in enumerate(waits[:-1]):
                        nop = mybir.InstNoOp(name=f"{inst.name}_sw{k}",
                                             engine=inst.engine, ins=[], outs=[])
                        nop.sync_info = mybir.SyncInfo(on_wait=[w], on_update=[])
                        out.append(nop)
                        n += 1
                    inst.sync_info = mybir.SyncInfo(
                        on_wait=[waits[-1]], on_update=list(si.on_update))
                    changed = True
                out.append(inst)
            if changed:
                b.instructions = out
    return n


# ------------------------------------------------------------------ builder
def _build(mm_dt="f32r", flags=None):
    """Emit the Bass program.  Returns (nc, input_names)."""
    _ensure_env()
    import concourse.bass as bass
    import concourse.mybir as mybir
    import concourse.tile as tile
    fl = flags or {}
    F32 = mybir.dt.float32
    BF16 = mybir.dt.bfloat16
    # fp32r for the small fp32-ish helpers (bias seeds, softmax recip rows)
    MMDT = mybir.dt.float32r

    nc = bass.Bass()

    # ---------------- dram tensors
    x0_d = nc.dram_tensor("x0", [T, D], F32, kind="ExternalInput")
    ident_d = nc.dram_tensor("ident", [P, P], F32, kind="ExternalInput")
    ones_d = nc.dram_tensor("ones", [1, P], MMDT, kind="ExternalInput")
    sel8_d = nc.dram_tensor("sel8", [P, 2 * P], MMDT, kind="ExternalInput")
    encT_d = nc.dram_tensor("encT", [P, NF, T], BF16, kind="ExternalInput")
    causal_d = nc.dram_tensor("causalD", [P, 2 * P], BF16,
                              kind="ExternalInput")
    keep_dec_d = nc.dram_tensor("keep_dec", [P, NT], F32, kind="ExternalInput")
    keep_enc_d = nc.dram_tensor("keep_enc", [P, NT], F32, kind="ExternalInput")
    out_d = nc.dram_tensor("out", [T, D], F32, kind="ExternalOutput")

    wname = []
    wd = {}

    def dram_w(name, shape):
        wd[name] = nc.dram_tensor(name, shape, BF16, kind="ExternalInput")
        wname.append(name)

    dram_w("sub_w1", [D, FF]); dram_w("sub_w2", [FF, D])
    for l in range(L):
        for pre in ("sa", "ca"):
            for wn in ("wq", "wk", "wv", "wo"):
                dram_w(f"{pre}_{wn}_{l}", [D, D])
        dram_w(f"ff_w1_{l}", [D, FF]); dram_w(f"ff_w2_{l}", [FF, D])
    if fl.get("any_rows"):
        wd["rows"] = nc.dram_tensor("rows", [fl["n_rows"], D], MMDT,
                                    kind="ExternalInput")
        wname.append("rows")
    if fl.get("any_cols"):
        wd["cols"] = nc.dram_tensor("cols", [P, fl["n_cols"]], F32,
                                    kind="ExternalInput")
        wname.append("cols")

    AF = mybir.ActivationFunctionType
    OP = mybir.AluOpType

    with tile.TileContext(nc) as tc:
        cpool = tc.alloc_tile_pool(name="consts", bufs=1)
        pool = tc.alloc_tile_pool(name="work", bufs=2)
        spool = tc.alloc_tile_pool(name="stats", bufs=8)
        ppool = tc.alloc_tile_pool(name="ps", bufs=2, space="PSUM")

        ident = cpool.tile([P, P], F32, name="ident_sb")
        nc.sync.dma_start(ident[:], ident_d[:])
        causalD = cpool.tile([P, 2 * P], BF16, name="causalD")
        nc.sync.dma_start(causalD[:], causal_d[:])
        keep_dec = cpool.tile([P, NT], F32, name="keep_dec_sb")
        nc.sync.dma_start(keep_dec[:], keep_dec_d[:])
        keep_enc = cpool.tile([P, NT], F32, name="keep_enc_sb")
        nc.sync.dma_start(keep_enc[:], keep_enc_d[:])
        ones_row = cpool.tile([1, P], MMDT, name="ones_row")
        nc.sync.dma_start(ones_row[:], ones_d[:])
        sel8 = cpool.tile([P, 2 * P], MMDT, name="sel8_sb")
        nc.sync.dma_start(sel8[:], sel8_d[:])
        eps_col = cpool.tile([P, 1], F32, name="eps_col")
        nc.vector.memset(eps_col[:], EPS)
        # full encoder output, feature-major, resident all 6 layers
        encT_sb = cpool.tile([P, NF, T], BF16, name="encT_sb")
        nc.sync.dma_start(encT_sb[:], encT_d[:])
        rows_sb = cols_sb = None
        if fl.get("any_rows"):
            rows_sb = cpool.tile([fl["n_rows"], D], MMDT, name="rows_sb")
            nc.sync.dma_start(rows_sb[:], wd["rows"][:])
        if fl.get("any_cols"):
            cols_sb = cpool.tile([P, fl["n_cols"]], F32, name="cols_sb")
            nc.sync.dma_start(cols_sb[:], wd["cols"][:])

        def load_w_proj(name):
            w = pool.tile([P, NF, D], BF16, tag="w", bufs=6, name="w_" + name)
            nc.sync.dma_start(w[:], wd[name].rearrange("(k p) d -> p k d", p=P))
            return w

        def psum_s(name):
            return ppool.tile([P, 1024], F32, tag="S", bufs=2, name=name)

        def psum_c(name):
            return ppool.tile([P, 1024], F32, tag="C", bufs=2, name=name)

        def seed_or_start(ps, row_idx, kt):
            """Start flag for the kt-th accumulation matmul; optionally seed
            the psum with a broadcast bias row first (K=1 matmul)."""
            if row_idx is not None and kt == 0:
                nc.tensor.matmul(
                    ps, ones_row[0:1, 0:ps.shape[0]],
                    rows_sb[row_idx:row_idx + 1, :ps.shape[-1]],
                    start=True, stop=False)
                return False
            return kt == 0

        # ---------------- building blocks --------------------------------
        def transpose_tp(x_sb, b, tp, xT, name):
            """transpose one token-tile pair of batch b into xT."""
            ps = psum_c(f"{name}_ps{tp}")
            for j in range(2):
                t = tp * 2 + j
                for f in range(NF):
                    nc.tensor.transpose(
                        ps[:, j * 512 + f * P: j * 512 + (f + 1) * P],
                        x_sb[:, b * NTB + t, f * P:(f + 1) * P],
                        ident[:],
                    )
            for j in range(2):
                t = tp * 2 + j
                nc.vector.tensor_copy(
                    out=xT[:, :, t * P:(t + 1) * P],
                    in_=ps[:, j * 512:(j + 1) * 512]
                    .rearrange("p (f q) -> p f q", q=P),
                )

        def transpose_b(x_sb, b, name):
            """token-major x tile [P, NT, D] (batch b) -> feature-major
            bf16 xT_b [P, NF, S]."""
            xT = pool.tile([P, NF, S], BF16, tag="xT", bufs=2, name=name)
            for tp in range(NTB // 2):
                transpose_tp(x_sb, b, tp, xT, name)
            return xT

        def proj_fp(srcT, w, yT, fp, col_base, copy_eng, name):
            """one output-feature pair of y^T = (src @ w)^T."""
            ps = psum_s(f"{name}_ps{fp}")
            for j in range(2):
                f = fp * 2 + j
                for kt in range(NF):
                    nc.tensor.matmul(
                        ps[:, j * 512:(j + 1) * 512],
                        w[:, kt, f * P:(f + 1) * P], srcT[:, kt, :],
                        start=(kt == 0), stop=(kt == NF - 1))
            if col_base is not None:
                for j in range(2):
                    f = fp * 2 + j
                    nc.scalar.activation(
                        yT[:, f, :], ps[:, j * 512:(j + 1) * 512],
                        AF.Identity,
                        bias=cols_sb[:, col_base + f:col_base + f + 1])
            elif copy_eng == "act":
                nc.scalar.copy(yT[:, fp * 2:fp * 2 + 2, :],
                               ps.rearrange("p (j q) -> p j q", q=512))
            else:
                nc.vector.tensor_copy(
                    out=yT[:, fp * 2:fp * 2 + 2, :],
                    in_=ps.rearrange("p (j q) -> p j q", q=512))

        def projv_tp(srcT, w, v, keep, b, tp, row_idx, name):
            """one token-tile pair of the V projection (with ones-columns)."""
            ps = psum_c(f"{name}_ps{tp}")
            for j2 in range(2):
                t = tp * 2 + j2
                half = ps[:, j2 * 512:(j2 + 1) * 512]
                for kt in range(NF):
                    st = seed_or_start(half, row_idx, kt)
                    nc.tensor.matmul(
                        half, srcT[:, kt, t * P:(t + 1) * P], w[:, kt, :],
                        start=st, stop=(kt == NF - 1))
            for j2 in range(2):
                t = tp * 2 + j2
                half = ps[:, j2 * 512:(j2 + 1) * 512]
                kcol = keep[:, b * NTB + t:b * NTB + t + 1]
                vv = v[:, t, :].rearrange("p (h c) -> p h c", c=65)
                nc.scalar.activation(vv[:, :, 0:64],
                                     half.rearrange("p (h c) -> p h c", c=64),
                                     AF.Copy, scale=kcol)
                nc.vector.tensor_copy(out=vv[:, :, 64:65],
                                      in_=kcol.to_broadcast((P, H, 1)))

        def ln_chain(ps, x_old, x_new, tt, step, name):
            """x_new[tt] = LN(ps + x_old[tt]); one-pass bn_stats reductions,
            rstd = exp(-0.5*ln(var+eps)) keeps ACT inside the exp table."""
            nc.vector.tensor_add(out=x_new[:, tt, :], in0=ps,
                                 in1=x_old[:, tt, :])
            st6 = spool.tile([P, 6], F32, tag="st6", bufs=8, name=name + "_s")
            nc.vector.bn_stats(st6[:], x_new[:, tt, :])
            mv = spool.tile([P, 2], F32, tag="st2", bufs=8, name=name + "_m")
            nc.vector.bn_aggr(mv[:], st6[:])
            lnv = spool.tile([P, 1], F32, tag="st1", bufs=8, name=name + "_l")
            nc.scalar.activation(lnv[:], mv[:, 1:2], AF.Ln, bias=eps_col[:])
            rstd = spool.tile([P, 1], F32, tag="st1", bufs=8, name=name + "_r")
            nc.scalar.activation(rstd[:], lnv[:], AF.Exp, scale=-0.5)
            nc.vector.tensor_scalar(
                out=x_new[:, tt, :], in0=x_new[:, tt, :],
                scalar1=mv[:, 0:1], scalar2=rstd[:],
                op0=OP.subtract, op1=OP.mult)
            gi = fl.get(f"g_{step}")
            if gi is not None:
                gb = _row_bcast(gi, name + "_g")
                nc.vector.tensor_mul(out=x_new[:, tt, :],
                                     in0=x_new[:, tt, :], in1=gb[0:P, :])
            bi = fl.get(f"beta_{step}")
            if bi is not None:
                bb = _row_bcast(bi, name + "_b")
                nc.vector.tensor_add(out=x_new[:, tt, :],
                                     in0=x_new[:, tt, :], in1=bb[0:P, :])

        def _row_bcast(row_idx, name):
            ps = psum_c(name + "_ps")
            nc.tensor.matmul(ps[:, 0:512], ones_row[0:1, 0:P],
                             rows_sb[row_idx:row_idx + 1, :],
                             start=True, stop=True)
            sb = pool.tile([P, 512], F32, tag="gb", bufs=2, name=name + "_sb")
            nc.scalar.copy(sb[:], ps[:, 0:512])
            return sb

        def make_cross_kv(l):
            """K/V projections of the (layer-constant) encoder output for
            cross-attention batch 0 — they depend only on encT, so they are
            donated as riders into the PRECEDING self-attention's second
            head loop (which otherwise starves the PE)."""
            wk = load_w_proj(f"ca_wk_{l}")
            wv = load_w_proj(f"ca_wv_{l}")
            kT = pool.tile([P, NF, S], BF16, tag="kT", bufs=2,
                           name=f"kT_cross_{l}_0")
            vv = pool.tile([P, NTB, H * 65], BF16, tag="v", bufs=2,
                           name=f"v_cross_{l}_0")
            srcT = encT_sb[:, :, 0:S]
            chunks = []
            for fp in range(2):
                chunks.append(lambda fp=fp: proj_fp(
                    srcT, wk, kT, fp, fl.get(f"bk_cross_{l}"), "act",
                    f"kT_cross_{l}_0"))
            for tp in range(2):
                chunks.append(lambda tp=tp: projv_tp(
                    srcT, wv, vv, keep_enc, 0, tp,
                    fl.get(f"bv_cross_{l}"), f"v_cross_{l}_0"))
            return chunks, {"wk": wk, "wv": wv, "kT": kT, "vv": vv}

        def attention(x_sb, x_new, wq_n, wk_n, wv_n, wo_n, cross, l,
                      pre_kv=None, extra_riders=None):
            """One attention sub-block (self or cross) + residual + LN.
            Emission is phase-staggered: batch-1 prep (PE-heavy) rides inside
            batch-0's head loop (ACT/DVE-heavy), and batch-0's tail rides
            inside batch-1's head loop."""
            tagp = "cross" if cross else "self"
            wq = load_w_proj(wq_n)
            if pre_kv is not None:
                wk, wv = pre_kv["wk"], pre_kv["wv"]
            else:
                wk = load_w_proj(wk_n)
                wv = load_w_proj(wv_n)
            keep = keep_enc if cross else keep_dec
            D_qT, D_kT, D_vv, D_ctxT, D_dns, D_expT = {}, {}, {}, {}, {}, {}
            ctx_ps = {}

            def prep_chunks(b):
                donated = pre_kv is not None and b == 0
                xT = pool.tile([P, NF, S], BF16, tag="xT", bufs=2,
                               name=f"xT_{tagp}_{l}_{b}")
                qT = pool.tile([P, NF, S], BF16, tag="qT", bufs=2,
                               name=f"qT_{tagp}_{l}_{b}")
                if donated:
                    kT, vv = pre_kv["kT"], pre_kv["vv"]
                else:
                    kT = pool.tile([P, NF, S], BF16, tag="kT", bufs=2,
                                   name=f"kT_{tagp}_{l}_{b}")
                    vv = pool.tile([P, NTB, H * 65], BF16, tag="v", bufs=2,
                                   name=f"v_{tagp}_{l}_{b}")
                D_qT[b] = qT; D_kT[b] = kT; D_vv[b] = vv
                D_ctxT[b] = pool.tile([P, NF, S], BF16, tag="ctxT", bufs=2,
                                      name=f"ctxT_{tagp}_{l}_{b}")
                srcT = encT_sb[:, :, b * S:(b + 1) * S] if cross else xT
                nq, nk, nv = (f"qT_{tagp}_{l}_{b}", f"kT_{tagp}_{l}_{b}",
                              f"v_{tagp}_{l}_{b}")

                def setup():
                    dns = []
                    for g in range(2):
                        dng = pool.tile([P, 512], F32, tag="dncol", bufs=4,
                                        name=f"dn_{tagp}_{l}_{b}_{g}")
                        nc.gpsimd.memset(dng[:], 1.0)
                        dns.append(dng)
                    D_dns[b] = dns
                    if not cross:
                        bufs_ = []
                        for par in range(2):
                            e = pool.tile([P, NTB, S], BF16, tag="expT",
                                          bufs=4, name=f"expT_{l}_{b}_{par}")
                            for kt in range(1, NTB):
                                nc.gpsimd.memset(e[:, kt, 0:kt * P], 0.0)
                            bufs_.append(e)
                        D_expT[b] = bufs_

                chunks = [setup]
                for tp in range(2):
                    chunks.append(lambda tp=tp: transpose_tp(
                        x_sb, b, tp, xT, f"xT_{tagp}_{l}_{b}"))
                for fp in range(2):
                    chunks.append(lambda fp=fp: proj_fp(
                        xT, wq, qT, fp, fl.get(f"bq_{tagp}_{l}"), "act", nq))
                if not donated:
                    for fp in range(2):
                        chunks.append(lambda fp=fp: proj_fp(
                            srcT, wk, kT, fp, fl.get(f"bk_{tagp}_{l}"),
                            "act", nk))
                    for tp in range(2):
                        chunks.append(lambda tp=tp: projv_tp(
                            srcT, wv, vv, keep, b, tp,
                            fl.get(f"bv_{tagp}_{l}"), nv))
                return chunks

            def emit_scores_self(b, h):
                qT, kT = D_qT[b], D_kT[b]
                po = (h % 2) * 64
                ft = h // 2
                expT = D_expT[b][h % 2]
                for pair in range(2):
                    ps = psum_c(f"s_{tagp}_{l}_{b}_{h}_{pair}")
                    for j in range(2):
                        kt = pair * 2 + j
                        n = S - kt * P
                        nc.tensor.matmul(
                            ps[:, j * 512:j * 512 + n],
                            kT[po:po + 64, ft, kt * P:(kt + 1) * P],
                            qT[po:po + 64, ft, kt * P:],
                            start=True, stop=True)
                    # diagonal-block causal mask for both kt in one DVE op
                    psd = ps.rearrange("p (j q) -> p j q", q=512)[:, :, 0:P]
                    nc.vector.tensor_tensor(
                        out=psd, in0=psd,
                        in1=causalD.rearrange("p (j q) -> p j q", q=P),
                        op=OP.add)
                    for j in range(2):
                        kt = pair * 2 + j
                        n = S - kt * P
                        nc.scalar.activation(
                            expT[:, kt, kt * P:],
                            ps[:, j * 512:j * 512 + n], AF.Exp)
                return expT

            def emit_scores_cross(b, h):
                qT, kT = D_qT[b], D_kT[b]
                po = (h % 2) * 64
                ft = h // 2
                expT = pool.tile([P, NTB, S], BF16, tag="expT", bufs=4,
                                 name=f"expT_c_{l}_{b}_{h}")
                for pair in range(2):
                    ps = psum_c(f"s_{tagp}_{l}_{b}_{h}_{pair}")
                    for j in range(2):
                        kt = pair * 2 + j
                        nc.tensor.matmul(
                            ps[:, j * 512:(j + 1) * 512],
                            kT[po:po + 64, ft, kt * P:(kt + 1) * P],
                            qT[po:po + 64, ft, :],
                            start=True, stop=True)
                    nc.scalar.activation(
                        expT[:, pair * 2:pair * 2 + 2, :],
                        ps.rearrange("p (j q) -> p j q", q=512), AF.Exp)
                return expT

            def emit_ctx(b, h, expT):
                vv, ctxT, dns = D_vv[b], D_ctxT[b], D_dns[b]
                po = (h % 2) * 64
                ft = h // 2
                if h % 2 == 0:
                    ctx_ps[b] = psum_s(f"c2_{tagp}_{l}_{b}_{h}")
                psc = ctx_ps[b][:, (h % 2) * 512:(h % 2 + 1) * 512]
                for kt in range(NTB):
                    nc.tensor.matmul(
                        psc[0:65, :],
                        vv[:, kt, h * 65:h * 65 + 65],
                        expT[:, kt, :],
                        start=(kt == 0), stop=(kt == NTB - 1))
                ro = 32 * (h % 4)
                nc.vector.tensor_copy(out=dns[h // 4][ro:ro + 1, :],
                                      in_=psc[64:65, :])
                nc.vector.tensor_copy(out=ctxT[po:po + 64, ft, :],
                                      in_=psc[0:64, :])

            wo = load_w_proj(wo_n)
            row_idx = fl.get(f"bo_{tagp}_{l}")

            def rc_half(b, g):
                """1/dn = exp(-ln(dn)) for head group g of batch b +
                normalize its two head-pairs.  Runs on ACT (both funcs live
                in the one loaded table) — the exact DVE reciprocal is ~6x
                slower and head-of-line-blocks the vector queue."""
                lnd = spool.tile([P, 512], F32, tag="lnd", bufs=2,
                                 name=f"lnd_{tagp}_{l}_{b}_{g}")
                nc.scalar.activation(lnd[:], D_dns[b][g][:], AF.Ln)
                rcg = pool.tile([P, 512], MMDT, tag="rc", bufs=4,
                                name=f"rc_{tagp}_{l}_{b}_{g}")
                nc.scalar.activation(rcg[:], lnd[:], AF.Exp, scale=-1.0)
                for jj in range(2):
                    hp = 2 * g + jj
                    psr = psum_c(f"rb_{tagp}_{l}_{b}_{hp}")
                    nc.tensor.matmul(psr[:, 0:512],
                                     sel8[:, jj * P:(jj + 1) * P],
                                     rcg[:], start=True, stop=True)
                    nc.vector.tensor_mul(out=D_ctxT[b][:, hp, :],
                                         in0=D_ctxT[b][:, hp, :],
                                         in1=psr[:, 0:512])

            def tail_chunks(b):
                """o-proj + residual + LN for batch b."""
                def oproj(tp):
                    ctxT = D_ctxT[b]
                    ps = psum_s(f"o_{tagp}_{l}_{b}_{tp}")
                    for j in range(2):
                        t = tp * 2 + j
                        half = ps[:, j * 512:(j + 1) * 512]
                        for ft2 in range(NF):
                            st = seed_or_start(half, row_idx, ft2)
                            nc.tensor.matmul(
                                half, ctxT[:, ft2, t * P:(t + 1) * P],
                                wo[:, ft2, :],
                                start=st, stop=(ft2 == NF - 1))
                    for j in range(2):
                        t = tp * 2 + j
                        ln_chain(ps[:, j * 512:(j + 1) * 512], x_sb, x_new,
                                 b * NTB + t, f"{tagp}_{l}",
                                 f"ln_{tagp}_{l}_{b}_{t}")

                return [lambda tp=tp: oproj(tp) for tp in range(NTB // 2)]

            emit_scores = emit_scores_cross if cross else emit_scores_self

            def head_loop(b, rider_slots):
                """scores/ctx pipeline for batch b; rider_slots[i] chunks
                (other batch's prep or tail) are emitted right after
                ctx(b, i) — slots are chosen so psum-ring reuse by riders
                never blocks a not-yet-emitted ctx (PE queue inversion)."""
                prev = None
                for h in range(H):
                    e = emit_scores(b, h)
                    if prev is not None:
                        emit_ctx(*prev)
                        if prev[1] == 3:
                            rc_half(b, 0)
                        for f in rider_slots.get(prev[1], ()):
                            f()
                    prev = (b, h, e)
                emit_ctx(*prev)
                rc_half(b, 1)
                for f in rider_slots.get(7, ()):
                    f()

            def head_loops():
                c1 = prep_chunks(1)
                # setup/transposes early; S-psum users (q/k proj) only in
                # slots where the previous ctx pair's psum has been freed
                head_loop(0, {0: [c1[0], c1[1]], 1: [c1[2]], 2: [c1[3]],
                              3: [c1[7]], 4: [c1[4]], 5: [c1[8]],
                              6: [c1[5]], 7: [c1[6]]})
                t0 = tail_chunks(0)
                slots1 = {2: [t0[0]], 4: [t0[1]]}
                if extra_riders:
                    xk0, xk1, xv0, xv1 = extra_riders
                    slots1[1] = [xk0]
                    slots1[3] = [xk1]
                    slots1[5] = [xv0]
                    slots1[6] = [xv1]
                head_loop(1, slots1)

            return prep_chunks(0) + [head_loops] + tail_chunks(1)

        def ffn(x_sb, x_new, w1_n, w2_n, l, relu_out, with_ln,
                b1_col_base=None, b2_row=None, store_out=None):
            """x_new = [LN](relu(x@w1+b1)@w2 + b2 [+x]) as a chunk list.
            Weights loaded once per layer; h1 for both batches per w1 chunk;
            y2 accumulated per token-tile-pair over all 16 h1 chunks."""
            w2all = pool.tile([P, NC16, D], BF16, tag="w2all", bufs=1,
                              name=f"w2all_{l}")
            xTs, h1s = {}, {}
            for b in range(BPC):
                xTs[b] = pool.tile([P, NF, S], BF16, tag="xT", bufs=2,
                                   name=f"xT_ffn_{l}_{b}")
                h1s[b] = pool.tile([P, NC16, S], BF16, tag="h1", bufs=2,
                                   name=f"h1_{l}_{b}")

            def setup():
                nc.sync.dma_start(
                    w2all[:], wd[w2_n].rearrange("(c p) d -> p c d", p=P))

            def hc_chunk(hc):
                w1c = pool.tile([P, NF, 512], BF16, tag="wf", bufs=2,
                                name=f"w1_{l}_{hc}")
                nc.sync.dma_start(
                    w1c[:], wd[w1_n].rearrange("(k p) d -> p k d", p=P)
                    [:, :, hc * 512:(hc + 1) * 512])
                for b in range(BPC):
                    for fp in range(2):
                        ph = psum_s(f"h_{l}_{b}_{hc}_{fp}")
                        for j in range(2):
                            f = fp * 2 + j
                            for kt in range(NF):
                                nc.tensor.matmul(
                                    ph[:, j * 512:(j + 1) * 512],
                                    w1c[:, kt, f * P:(f + 1) * P],
                                    xTs[b][:, kt, :],
                                    start=(kt == 0), stop=(kt == NF - 1))
                        c0 = hc * NF + fp * 2
                        if b1_col_base is not None:
                            for j in range(2):
                                cb = c0 + j
                                nc.scalar.activation(
                                    h1s[b][:, c0 + j, :],
                                    ph[:, j * 512:(j + 1) * 512], AF.Relu,
                                    bias=cols_sb[:, b1_col_base + cb:
                                                 b1_col_base + cb + 1])
                        elif fp == 0:
                            nc.scalar.activation(
                                h1s[b][:, c0:c0 + 2, :],
                                ph.rearrange("p (j q) -> p j q", q=512),
                                AF.Relu)
                        else:
                            nc.vector.tensor_scalar_max(
                                h1s[b][:, c0:c0 + 2, :],
                                ph.rearrange("p (j q) -> p j q", q=512), 0.0)

            def y2_chunk(b, tp):
                psy = psum_s(f"y2_{l}_{b}_{tp}")
                for j2 in range(2):
                    t = tp * 2 + j2
                    half = psy[:, j2 * 512:(j2 + 1) * 512]
                    for c in range(NC16):
                        st = (c == 0)
                        if st and b2_row is not None:
                            st = seed_or_start(half, b2_row, 0)
                        nc.tensor.matmul(
                            half, h1s[b][:, c, t * P:(t + 1) * P],
                            w2all[:, c, :],
                            start=st, stop=(c == NC16 - 1))
                for j2 in range(2):
                    t = tp * 2 + j2
                    tt = b * NTB + t
                    half = psy[:, j2 * 512:(j2 + 1) * 512]
                    if relu_out:
                        if j2 == 0:
                            nc.scalar.activation(x_new[:, tt, :], half,
                                                 AF.Relu)
                        else:
                            nc.vector.tensor_scalar_max(
                                x_new[:, tt, :], half, 0.0)
                    elif with_ln:
                        ln_chain(half, x_sb, x_new, tt, f"ff_{l}",
                                 f"lnf_{l}_{b}_{t}")
                    else:
                        nc.scalar.copy(x_new[:, tt, :], half)
                    if store_out is not None:
                        nc.sync.dma_start(store_out[:, tt, :],
                                          x_new[:, tt, :])

            chunks = [setup]
            for b in range(BPC):
                for tp in range(NTB // 2):
                    chunks.append(lambda b=b, tp=tp: transpose_tp(
                        x_sb, b, tp, xTs[b], f"xT_ffn_{l}_{b}"))
            for hc in range(NHC):
                chunks.append(lambda hc=hc: hc_chunk(hc))
            for b in range(BPC):
                for tp in range(NTB // 2):
                    chunks.append(lambda b=b, tp=tp: y2_chunk(b, tp))
            return chunks

        # ---------------- program ----------------------------------------
        # Each sublayer is a chunk list; consecutive sublayers are riffled
        # at the boundary (new sublayer's early chunks land BEFORE the old
        # one's final LN/o-proj chunks in every engine queue) so the PE
        # always has next-phase matmuls during tail element-wise work.
        x = pool.tile([P, NT, D], F32, tag="x", bufs=2, name="x_in")
        for tt in range(NT):
            nc.sync.dma_start(
                x[:, tt, :],
                x0_d.rearrange("(t p) d -> p t d", p=P)[:, tt, :])

        OVL = 3
        carry = []

        def run_stream():
            nonlocal carry
            for chunks in gen:
                k = min(len(carry), len(chunks) - OVL)
                for i in range(k):
                    chunks[i]()
                    carry[i]()
                for f in carry[k:]:
                    f()
                body = chunks[k:]
                for f in body[:len(body) - OVL]:
                    f()
                carry = body[len(body) - OVL:]
            for f in carry:
                f()

        xs = {"cur": x}

        def gen_lists():
            x1 = pool.tile([P, NT, D], F32, tag="x", bufs=2, name="x_sub")
            yield ffn(xs["cur"], x1, "sub_w1", "sub_w2", "sub",
                      relu_out=True, with_ln=False,
                      b1_col_base=fl.get("b1_sub"), b2_row=fl.get("b2_sub"))
            xs["cur"] = x1
            for l in range(L):
                kv_chunks, kv_tiles = make_cross_kv(l)
                xa = pool.tile([P, NT, D], F32, tag="x", bufs=2,
                               name=f"x_sa_{l}")
                yield attention(xs["cur"], xa, f"sa_wq_{l}", f"sa_wk_{l}",
                                f"sa_wv_{l}", f"sa_wo_{l}", cross=False, l=l,
                                extra_riders=kv_chunks)
                xs["cur"] = xa
                xb = pool.tile([P, NT, D], F32, tag="x", bufs=2,
                               name=f"x_ca_{l}")
                yield attention(xs["cur"], xb, f"ca_wq_{l}", f"ca_wk_{l}",
                                f"ca_wv_{l}", f"ca_wo_{l}", cross=True, l=l,
                                pre_kv=kv_tiles)
                xs["cur"] = xb
                xc = pool.tile([P, NT, D], F32, tag="x", bufs=2,
                               name=f"x_ff_{l}")
                yield ffn(xs["cur"], xc, f"ff_w1_{l}", f"ff_w2_{l}", l,
                          relu_out=False, with_ln=True,
                          b1_col_base=fl.get(f"b1_ff_{l}"),
                          b2_row=fl.get(f"b2_ff_{l}"),
                          store_out=(out_d.rearrange("(t p) d -> p t d", p=P)
                                     if l == L - 1 else None))
                xs["cur"] = xc

        gen = gen_lists()
        run_stream()

        ppool.release(); spool.release(); pool.release(); cpool.release()

    _split_multi_waits(nc)

    names = ["x0", "ident", "ones", "sel8", "encT", "causalD", "keep_dec",
             "keep_enc"] + wname
    return nc, names


# -------------------------------------------------------------------- host
def _host_prep(inputs):
    """Returns (per-core input maps, build flags)."""
    import ml_dtypes
    BF = ml_dtypes.bfloat16
    npa = {k: np.asarray(v) for k, v in inputs.items()}
    dec = npa["dec_inputs"]          # [B, S] int
    enc_in = npa["enc_inputs"]       # [B, S] int
    enc_out = np.ascontiguousarray(npa["enc_outputs"], dtype=np.float32)
    pad = int(npa["pad_ids"])
    emb = npa["emb"].astype(np.float32, copy=False)
    pe = npa["pe"].astype(np.float32, copy=False)

    x0 = emb[dec] + pe[None, :S]                       # [B, S, D]
    x0 = np.ascontiguousarray(x0, dtype=np.float32)

    # one diagonal [k, q] block of the causal mask (identical for every tile)
    ii = np.arange(P)
    causalD = np.where(ii[None, :] >= ii[:, None], 0.0, -1e9).astype(BF)
    causalD = np.concatenate([causalD, causalD], axis=1)     # [P, 2*P]

    keep_dec = (dec != pad).astype(np.float32)         # [B, S]
    keep_enc = (enc_in != pad).astype(np.float32)

    flags = {}
    sel8 = np.zeros((P, 2 * P), dtype=np.float32)
    for j in range(2):
        sel8[64 * j, j * P:j * P + 64] = 1.0
        sel8[64 * j + 32, j * P + 64:j * P + P] = 1.0
    shared = {"causalD": causalD,
              "ident": np.eye(P, dtype=np.float32),
              "ones": np.ones((1, P), dtype=np.float32),
              "sel8": sel8}
    shared["sub_w1"] = npa["sub_w1"].astype(BF)
    shared["sub_w2"] = npa["sub_w2"].astype(BF)
    for l in range(L):
        shared[f"sa_wq_{l}"] = (npa["sa_wq"][l] / np.sqrt(DK)).astype(BF)
        shared[f"ca_wq_{l}"] = (npa["ca_wq"][l] / np.sqrt(DK)).astype(BF)
        for pre in ("sa", "ca"):
            for wn in ("wk", "wv", "wo"):
                shared[f"{pre}_{wn}_{l}"] = npa[f"{pre}_{wn}"][l].astype(BF)
        shared[f"ff_w1_{l}"] = npa["ff_w1"][l].astype(BF)
        shared[f"ff_w2_{l}"] = npa["ff_w2"][l].astype(BF)

    # ---- optional bias / gain handling (all trivial for this model's
    # setup_inputs, so normally nothing extra is emitted) ------------------
    rows, cols = [], []

    def add_row(arr, key):
        if np.any(arr != 0.0):
            flags[key] = len(rows)
            rows.append(np.asarray(arr, dtype=np.float32))

    def add_cols(arr, key):
        if np.any(arr != 0.0):
            flags[key] = len(cols)
            c = np.asarray(arr, dtype=np.float32).reshape(-1, P).T  # [P, n]
            for i in range(c.shape[1]):
                cols.append(c[:, i])

    def add_gain(g_arr, b_arr, step):
        if np.any(g_arr != 1.0):
            flags[f"g_{step}"] = len(rows)
            rows.append(np.asarray(g_arr, dtype=np.float32))
        if np.any(b_arr != 0.0):
            flags[f"beta_{step}"] = len(rows)
            rows.append(np.asarray(b_arr, dtype=np.float32))

    add_cols(npa["sub_b1"], "b1_sub")
    add_row(npa["sub_b2"], "b2_sub")
    for l in range(L):
        add_cols(npa["sa_bq"][l] / np.sqrt(DK), f"bq_self_{l}")
        add_cols(npa["sa_bk"][l], f"bk_self_{l}")
        add_row(npa["sa_bv"][l], f"bv_self_{l}")
        add_row(npa["sa_bo"][l], f"bo_self_{l}")
        add_gain(npa["sa_g"][l], npa["sa_beta"][l], f"self_{l}")
        add_cols(npa["ca_bq"][l] / np.sqrt(DK), f"bq_cross_{l}")
        add_cols(npa["ca_bk"][l], f"bk_cross_{l}")
        add_row(npa["ca_bv"][l], f"bv_cross_{l}")
        add_row(npa["ca_bo"][l], f"bo_cross_{l}")
        add_gain(npa["ca_g"][l], npa["ca_beta"][l], f"cross_{l}")
        add_cols(npa["ff_b1"][l], f"b1_ff_{l}")
        add_row(npa["ff_b2"][l], f"b2_ff_{l}")
        add_gain(npa["ff_g"][l], npa["ff_beta"][l], f"ff_{l}")
    if rows:
        flags["any_rows"] = True
        flags["n_rows"] = len(rows)
        shared["rows"] = np.stack(rows)
    if cols:
        flags["any_cols"] = True
        flags["n_cols"] = len(cols)
        shared["cols"] = np.ascontiguousarray(np.stack(cols, axis=1))

    in_maps = []
    for c in range(NCORES):
        bs = slice(c * BPC, (c + 1) * BPC)
        m = dict(shared)
        m["x0"] = x0[bs].reshape(T, D)
        e = enc_out[bs].reshape(T, D)                      # [T, D]
        m["encT"] = np.ascontiguousarray(
            e.T.reshape(NF, P, T).transpose(1, 0, 2)).astype(BF)  # [P, NF, T]
        m["keep_dec"] = np.ascontiguousarray(
            keep_dec[bs].reshape(NT, P).T)                 # [P, NT]
        m["keep_enc"] = np.ascontiguousarray(
            keep_enc[bs].reshape(NT, P).T)
        in_maps.append(m)
    return in_maps, flags


_cache = {}


def run(inputs, mm_dt="f32r", trace=False):
    """Build (cached), run on 8 cores, gather.  Returns (out, results)."""
    _ensure_env()
    from concourse.bass_utils import run_bass_kernel_spmd

    in_maps, flags = _host_prep(inputs)
    key = (mm_dt, tuple(sorted(flags.items())))
    if key not in _cache:
        _cache[key] = _build(mm_dt=mm_dt, flags=flags)
    nc, names = _cache[key]
    res = run_bass_kernel_spmd(nc, in_maps, core_ids=list(range(NCORES)),
                               trace=trace)
    out = np.stack([r["out"] for r in res.results])        # [8, T, D]
    out = out.reshape(B, S, D)
    return out, res


def kernel(**inputs) -> np.ndarray:
    out, _ = run(inputs, mm_dt="f32r", trace=False)
    return out
